# revision 6
# baseline (speedup 1.0000x reference)
"""Trainium2 Bass kernel for a channel-attention block.

Reference math (per batch sample, a: [C, N] with C=128 channels,
N = H*W spatial):
    b   = a @ a.T                  # [C, C] channel affinity (Gram)
    x   = softmax(b, axis=-1)
    c   = x @ a                    # [C, N]
    out = beta * c + a

Sharding: data-parallel over the batch dim — 16 samples / 8 cores =
2 samples per NeuronCore, no cross-core communication.

Per-core pipeline (per sample):
  stage A: stream `a` in [128, 2048] f32 tiles; PE-transpose each
           128x128 block (n on partitions), ACT copy-casts PSUM->SBUF
           bf16, then bf16 Gram matmuls accumulate b in one PSUM bank.
  stage B: row softmax on b (DVE max, ACT exp(+bias) with fused row
           sum, DVE reciprocal, DVE scale), PE-transpose X -> XT.
  stage C: stream `a` again; c = XT.T @ a via fp32 matmuls (no cast
           pass needed; PE has headroom under the DMA roofline), fused
           DVE epilogue out = (c * beta) + a, store.

The kernel is DMA-bound: 3 passes x 32 MB per sample = 192 MB per
core at ~360 GB/s.
"""

import numpy as np

import concourse.bass as bass
import concourse.mybir as mybir
import concourse.tile as tile
from concourse import bacc
from concourse.bass_utils import run_bass_kernel_spmd
from concourse.masks import make_identity

F32 = mybir.dt.float32
F32R = mybir.dt.float32r
BF16 = mybir.dt.bfloat16

N_CORES = 8
B, C, H, W = 16, 128, 256, 256
N_FULL = H * W
S = B // N_CORES  # samples per core


def build(S=S, C=C, N=N_FULL, load=2048, stage_c="bf16", bufs=4):
    """Build + compile the per-core Bass program.

    Emission interleaves sample s's stage C with sample s+1's stage A so
    the (DMA-bound) attend phase overlaps the (PE-heavier) Gram phase.
    """
    assert C == 128 and N % load == 0 and load % 512 == 0
    nc = bacc.Bacc("TRN2", target_bir_lowering=False, debug=False)

    a_d = nc.dram_tensor("a", [S, C, N], F32, kind="ExternalInput").ap()
    beta_d = nc.dram_tensor("beta", [C, 1], F32, kind="ExternalInput").ap()
    out_d = nc.dram_tensor("out", [S, C, N], F32, kind="ExternalOutput").ap()

    n_loads = N // load
    TW = 512             # transpose-group width: 4 transposes per ACT copy
    MM_N = 512           # stage-C matmul moving width (1 PSUM bank f32)
    n_gram_mm = N // 128

    with tile.TileContext(nc) as tc:
        with (
            tc.tile_pool(name="const", bufs=1) as const_pool,
            tc.tile_pool(name="a_in", bufs=bufs) as a_pool,
            tc.tile_pool(name="at", bufs=4) as at_pool,
            tc.tile_pool(name="sm", bufs=2) as sm_pool,
            tc.tile_pool(name="cin", bufs=bufs) as cin_pool,
            tc.tile_pool(name="cout", bufs=bufs) as cout_pool,
            tc.tile_pool(name="tp_ps", bufs=3, space="PSUM") as tp_psum,
            tc.tile_pool(name="gram_ps", bufs=2, space="PSUM") as gram_psum,
            tc.tile_pool(name="c_ps", bufs=3, space="PSUM") as c_psum,
        ):
            ident = const_pool.tile([128, 128], F32, tag="ident")
            make_identity(nc, ident)
            beta_sb = const_pool.tile([C, 1], F32, tag="beta")
            nc.sync.dma_start(beta_sb, beta_d)

            gram_state = {}   # s -> (b_ps, mm_count)
            xt_w = {}         # s -> lhsT weights for stage C

            def stage_a_chunk(s, j):
                """Load tile j of sample s, transpose, Gram-accumulate."""
                if s not in gram_state:
                    b_ps = gram_psum.tile([C, C], F32, tag="gram",
                                          name=f"gram_{s}")
                    gram_state[s] = [b_ps, 0]
                st = gram_state[s]
                b_ps = st[0]
                a_t = a_pool.tile([C, load], F32, tag="a_in", name=f"a_{s}_{j}")
                nc.sync.dma_start(a_t, a_d[s, :, j * load:(j + 1) * load])
                for g in range(load // TW):
                    tp = tp_psum.tile([128, TW], F32, tag="tp",
                                      name=f"tp_{s}_{j}_{g}")
                    for q in range(TW // 128):
                        nc.tensor.transpose(
                            tp[:, q * 128:(q + 1) * 128],
                            a_t[:, g * TW + q * 128: g * TW + (q + 1) * 128],
                            ident,
                        )
                    at_t = at_pool.tile([128, TW], BF16, tag="at",
                                        name=f"at_{s}_{j}_{g}")
                    nc.scalar.copy(at_t, tp)
                    for q in range(TW // 128):
                        st[1] += 1
                        nc.tensor.matmul(
                            b_ps,
                            lhsT=at_t[:, q * 128:(q + 1) * 128],
                            rhs=at_t[:, q * 128:(q + 1) * 128],
                            start=(st[1] == 1),
                            stop=(st[1] == n_gram_mm),
                        )

            def softmax(s):
                """X = softmax(b) rows; store XT (transposed) for stage C."""
                b_ps = gram_state[s][0]
                negm = sm_pool.tile([C, 1], F32, tag="negm", name=f"negm_{s}")
                nc.vector.tensor_reduce(
                    negm, b_ps, axis=mybir.AxisListType.X,
                    op=mybir.AluOpType.max, negate=True,
                )
                e_t = sm_pool.tile([C, C], F32, tag="e", name=f"e_{s}")
                ssum = sm_pool.tile([C, 1], F32, tag="ssum", name=f"ssum_{s}")
                nc.scalar.activation(
                    e_t, b_ps, mybir.ActivationFunctionType.Exp,
                    bias=negm, accum_out=ssum,
                )
                rec = sm_pool.tile([C, 1], F32, tag="rec", name=f"rec_{s}")
                nc.vector.reciprocal(rec, ssum)
                x_t = sm_pool.tile([C, C], F32, tag="x", name=f"x_{s}")
                nc.vector.tensor_scalar_mul(x_t, e_t, rec)
                xt_ps = tp_psum.tile([128, TW], F32, tag="tp", name=f"xtp_{s}")
                nc.tensor.transpose(xt_ps[:, :128], x_t, ident)
                xt_dt = F32 if stage_c == "f32" else BF16
                xt_sb = sm_pool.tile([C, C], xt_dt, tag="xt", name=f"xt_{s}")
                nc.scalar.copy(xt_sb, xt_ps[:, :128])
                xt_w[s] = xt_sb

            def stage_c_chunk(s, j):
                """c = XT.T @ a on tile j; out = beta*c + a; store."""
                lhs_w = xt_w[s]
                c_in = cin_pool.tile([C, load], F32, tag="cin",
                                     name=f"cin_{s}_{j}")
                nc.sync.dma_start(c_in, a_d[s, :, j * load:(j + 1) * load])
                if stage_c == "bf16":
                    rhs_b = cin_pool.tile([C, load], BF16, tag="cinb",
                                          name=f"cinb_{s}_{j}")
                    nc.vector.tensor_copy(rhs_b, c_in)
                c_out = cout_pool.tile([C, load], F32, tag="cout",
                                       name=f"cout_{s}_{j}")
                for q in range(load // MM_N):
                    sl = slice(q * MM_N, (q + 1) * MM_N)
                    rhs_mm = rhs_b[:, sl] if stage_c == "bf16" else c_in[:, sl]
                    c_ps = c_psum.tile([128, MM_N], F32, tag="cps",
                                       name=f"cps_{s}_{j}_{q}")
                    nc.tensor.matmul(
                        c_ps, lhsT=lhs_w, rhs=rhs_mm, start=True, stop=True,
                    )
                    nc.vector.scalar_tensor_tensor(
                        out=c_out[:, sl],
                        in0=c_ps,
                        scalar=beta_sb,
                        in1=c_in[:, sl],
                        op0=mybir.AluOpType.mult,
                        op1=mybir.AluOpType.add,
                    )
                nc.scalar.dma_start(out_d[s, :, j * load:(j + 1) * load], c_out)

            # Software-pipelined emission across samples:
            #   A(0); sm(0); [C(0) x A(1)]; sm(1); [C(1) x A(2)]; ... C(S-1)
            for j in range(n_loads):
                stage_a_chunk(0, j)
            softmax(0)
            for s in range(1, S):
                for j in range(n_loads):
                    stage_c_chunk(s - 1, j)
                    stage_a_chunk(s, j)
                softmax(s)
            for j in range(n_loads):
                stage_c_chunk(S - 1, j)

    nc.compile()
    return nc


_NC_CACHE: dict = {}


def _get_nc(**kw):
    key = tuple(sorted(kw.items()))
    if key not in _NC_CACHE:
        _NC_CACHE[key] = build(**kw)
    return _NC_CACHE[key]


def kernel(a, beta):
    """Full-input entry point: a [16,128,256,256] f32, beta [1] f32."""
    a = np.ascontiguousarray(np.asarray(a, dtype=np.float32))
    beta = np.asarray(beta, dtype=np.float32)
    nb, ch, h, w = a.shape
    n = h * w
    s = nb // N_CORES
    a3 = a.reshape(nb, ch, n)
    beta_b = np.broadcast_to(beta.reshape(1, 1), (ch, 1)).copy()

    nc = _get_nc(S=s, C=ch, N=n)
    in_maps = [
        {"a": a3[i * s:(i + 1) * s], "beta": beta_b} for i in range(N_CORES)
    ]
    res = run_bass_kernel_spmd(nc, in_maps, list(range(N_CORES)))
    out = np.concatenate([res.results[i]["out"] for i in range(N_CORES)], axis=0)
    return out.reshape(nb, ch, h, w).astype(np.float32, copy=False)


# revision 31
# speedup vs baseline: 1.0919x; 1.0919x over previous
"""Trainium2 Bass kernel for a channel-attention block.

Reference math (per batch sample, a: [C, N] with C=128 channels,
N = H*W spatial):
    b   = a @ a.T                  # [C, C] channel affinity (Gram)
    x   = softmax(b, axis=-1)
    c   = x @ a                    # [C, N]
    out = beta * c + a

Sharding: data-parallel over the batch dim — 16 samples / 8 cores =
2 samples per NeuronCore, no cross-core communication.

Per-core pipeline (per sample):
  stage A: stream `a` in [128, 2048] f32 tiles; PE-transpose each
           128x128 block (n on partitions), ACT copy-casts PSUM->SBUF
           bf16, then bf16 Gram matmuls accumulate b in one PSUM bank.
  stage B: row softmax on b (DVE max, ACT exp(+bias) with fused row
           sum, DVE reciprocal, DVE scale), PE-transpose X -> XT.
  stage C: stream `a` again; c = XT.T @ a via fp32 matmuls (no cast
           pass needed; PE has headroom under the DMA roofline), fused
           DVE epilogue out = (c * beta) + a, store.

The kernel is DMA-bound: 3 passes x 32 MB per sample = 192 MB per
core at ~360 GB/s.
"""

import numpy as np

import concourse.bass as bass
import concourse.mybir as mybir
import concourse.tile as tile
from concourse import bacc
from concourse.bass_utils import run_bass_kernel_spmd
from concourse.masks import make_identity

F32 = mybir.dt.float32
F32R = mybir.dt.float32r
BF16 = mybir.dt.bfloat16

N_CORES = 8
B, C, H, W = 16, 128, 256, 256
N_FULL = H * W
S = B // N_CORES  # samples per core


def build(S=S, C=C, N=N_FULL, load=2048, stage_c="bf16", bufs=3, cache_k=14,
          precast=True, eng_precast="dve", eng_atcopy="alt", eng_ccast="dve"):
    """Build + compile the per-core Bass program.

    Emission interleaves sample s's stage C with sample s+1's stage A so
    the (DMA-bound) attend phase overlaps the (PE-heavier) Gram phase;
    stage A is emitted at 2x rate so its Gram finishes (and softmax runs)
    while C(s-1) still has work, hiding the softmax latency.
    The last `cache_k/2` stage-A input tiles of each sample stay resident
    in SBUF (two pools, alternating by sample parity, so slot reuse never
    stalls the next sample's fills) and stage C skips re-loading them.
    """
    assert C == 128 and N % load == 0 and load % 512 == 0
    cache_k = min(cache_k, N // load)
    nc = bacc.Bacc("TRN2", target_bir_lowering=False, debug=False)

    a_d = nc.dram_tensor("a", [S, C, N], F32, kind="ExternalInput").ap()
    beta_d = nc.dram_tensor("beta", [C, 1], F32, kind="ExternalInput").ap()
    out_d = nc.dram_tensor("out", [S, C, N], F32, kind="ExternalOutput").ap()

    n_loads = N // load
    TW = 512             # transpose-group width: 4 transposes per ACT copy
    MM_N = 512           # stage-C matmul moving width (1 PSUM bank f32)
    n_gram_mm = N // 128

    with tile.TileContext(nc) as tc:
        with (
            tc.tile_pool(name="const", bufs=1) as const_pool,
            tc.tile_pool(name="a_in", bufs=bufs) as a_pool,
            tc.tile_pool(name="at", bufs=4) as at_pool,
            tc.tile_pool(name="sm", bufs=2) as sm_pool,
            tc.tile_pool(name="cin", bufs=bufs) as cin_pool,
            tc.tile_pool(name="cout", bufs=bufs) as cout_pool,
            tc.tile_pool(name="acache", bufs=max(cache_k, 1)) as cache_pool,
            tc.tile_pool(name="tp_ps", bufs=3, space="PSUM") as tp_psum,
            tc.tile_pool(name="gram_ps", bufs=2, space="PSUM") as gram_psum,
            tc.tile_pool(name="c_ps", bufs=3, space="PSUM") as c_psum,
        ):
            ident = const_pool.tile([128, 128], F32, tag="ident")
            make_identity(nc, ident)
            ident_bf = const_pool.tile([128, 128], BF16, tag="identbf")
            make_identity(nc, ident_bf)
            beta_sb = const_pool.tile([C, 1], F32, tag="beta")
            nc.sync.dma_start(beta_sb, beta_d)

            def copy_op(engine_sel, idx, out, in_):
                """Route a copy/cast to ACT or DVE per engine_sel."""
                if engine_sel == "act" or (engine_sel == "alt" and idx % 2 == 0):
                    nc.scalar.copy(out, in_)
                else:
                    nc.vector.tensor_copy(out, in_)

            gram_state = {}   # s -> (b_ps, mm_count)
            xt_w = {}         # s -> lhsT weights for stage C
            beta_row = {}     # s -> beta/rowsum [C,1]
            cached = {}       # (s, j) -> SBUF-resident a tile

            def stage_a_chunk(s, j):
                """Load tile j of sample s, transpose, Gram-accumulate."""
                if s not in gram_state:
                    b_ps = gram_psum.tile([C, C], F32, tag="gram",
                                          name=f"gram_{s}")
                    gram_state[s] = [b_ps, 0]
                st = gram_state[s]
                b_ps = st[0]
                if j >= n_loads - cache_k:
                    a_t = cache_pool.tile([C, load], F32, tag="acache",
                                          name=f"ac_{s}_{j}")
                    cached[(s, j)] = a_t
                else:
                    a_t = a_pool.tile([C, load], F32, tag="a_in",
                                      name=f"a_{s}_{j}")
                nc.sync.dma_start(a_t, a_d[s, :, j * load:(j + 1) * load])
                for g in range(load // TW):
                    src = a_t[:, g * TW:(g + 1) * TW]
                    if precast:
                        abf = at_pool.tile([128, TW], BF16, tag="abf",
                                           name=f"abf_{s}_{j}_{g}")
                        copy_op(eng_precast, g, abf, src)
                        src = abf
                    t_dt = BF16 if precast else F32
                    t_id = ident_bf if precast else ident
                    tp = tp_psum.tile([128, TW], t_dt, tag="tp",
                                      name=f"tp_{s}_{j}_{g}")
                    for q in range(TW // 128):
                        nc.tensor.transpose(
                            tp[:, q * 128:(q + 1) * 128],
                            src[:, q * 128:(q + 1) * 128],
                            t_id,
                        )
                    at_t = at_pool.tile([128, TW], BF16, tag="at",
                                        name=f"at_{s}_{j}_{g}")
                    copy_op(eng_atcopy, g, at_t, tp)
                    for q in range(TW // 128):
                        st[1] += 1
                        nc.tensor.matmul(
                            b_ps,
                            lhsT=at_t[:, q * 128:(q + 1) * 128],
                            rhs=at_t[:, q * 128:(q + 1) * 128],
                            start=(st[1] == 1),
                            stop=(st[1] == n_gram_mm),
                        )

            def softmax(s):
                """Unnormalized softmax: E = exp(b - rowmax), transposed for
                stage C. The 1/rowsum normalization folds into the epilogue
                scalar bs = beta/rowsum, keeping the first stage-C matmul off
                the DVE reciprocal's critical path."""
                b_ps = gram_state[s][0]
                negm = sm_pool.tile([C, 1], F32, tag="negm", name=f"negm_{s}")
                nc.vector.tensor_reduce(
                    negm, b_ps, axis=mybir.AxisListType.X,
                    op=mybir.AluOpType.max, negate=True,
                )
                e_dt = F32 if stage_c == "f32" else BF16
                e_t = sm_pool.tile([C, C], e_dt, tag="e", name=f"e_{s}")
                ssum = sm_pool.tile([C, 1], F32, tag="ssum", name=f"ssum_{s}")
                nc.scalar.activation(
                    e_t, b_ps, mybir.ActivationFunctionType.Exp,
                    bias=negm, accum_out=ssum,
                )
                xt_ps = tp_psum.tile([128, TW], e_dt, tag="tp", name=f"xtp_{s}")
                nc.tensor.transpose(xt_ps[:, :128], e_t,
                                    ident if e_dt == F32 else ident_bf)
                xt_sb = sm_pool.tile([C, C], e_dt, tag="xt", name=f"xt_{s}")
                nc.scalar.copy(xt_sb, xt_ps[:, :128])
                xt_w[s] = xt_sb
                rec = sm_pool.tile([C, 1], F32, tag="rec", name=f"rec_{s}")
                nc.vector.reciprocal(rec, ssum)
                bs = sm_pool.tile([C, 1], F32, tag="bs", name=f"bs_{s}")
                nc.vector.tensor_scalar_mul(bs, rec, beta_sb)
                beta_row[s] = bs

            def stage_c_chunk(s, j):
                """c = XT.T @ a on tile j; out = beta*c + a; store."""
                lhs_w = xt_w[s]
                c_in = cached.pop((s, j), None)
                if c_in is None:
                    c_in = cin_pool.tile([C, load], F32, tag="cin",
                                         name=f"cin_{s}_{j}")
                    nc.sync.dma_start(c_in, a_d[s, :, j * load:(j + 1) * load])
                if stage_c == "bf16":
                    rhs_b = cin_pool.tile([C, load], BF16, tag="cinb",
                                          name=f"cinb_{s}_{j}")
                    copy_op(eng_ccast, j, rhs_b, c_in)
                c_out = cout_pool.tile([C, load], F32, tag="cout",
                                       name=f"cout_{s}_{j}")
                for q in range(load // MM_N):
                    sl = slice(q * MM_N, (q + 1) * MM_N)
                    rhs_mm = rhs_b[:, sl] if stage_c == "bf16" else c_in[:, sl]
                    c_ps = c_psum.tile([128, MM_N], F32, tag="cps",
                                       name=f"cps_{s}_{j}_{q}")
                    nc.tensor.matmul(
                        c_ps, lhsT=lhs_w, rhs=rhs_mm, start=True, stop=True,
                    )
                    nc.vector.scalar_tensor_tensor(
                        out=c_out[:, sl],
                        in0=c_ps,
                        scalar=beta_row[s],
                        in1=c_in[:, sl],
                        op0=mybir.AluOpType.mult,
                        op1=mybir.AluOpType.add,
                    )
                nc.scalar.dma_start(out_d[s, :, j * load:(j + 1) * load], c_out)

            # Stage C consumes its SBUF-cached tiles FIRST so the shared
            # cache slots free before the next sample's stage-A tail needs
            # to refill them.
            def c_order():
                return (list(range(n_loads - cache_k, n_loads))
                        + list(range(0, n_loads - cache_k)))

            # Software-pipelined emission across samples:
            #   A(0); sm(0); [A(1) 2x-rate x C(0)]; sm(1); C(0) tail; ...
            # Stage A runs at 2x emission rate so its Gram (and softmax)
            # complete while C(s-1) still has work queued.
            for j in range(n_loads):
                stage_a_chunk(0, j)
            softmax(0)
            for s in range(1, S):
                order = c_order()
                ci = 0
                for j in range(n_loads):
                    stage_a_chunk(s, j)
                    if j % 2 == 1:
                        stage_c_chunk(s - 1, order[ci])
                        ci += 1
                softmax(s)
                while ci < n_loads:
                    stage_c_chunk(s - 1, order[ci])
                    ci += 1
            for j in c_order():
                stage_c_chunk(S - 1, j)

    nc.compile()
    return nc


_NC_CACHE: dict = {}


def _get_nc(**kw):
    key = tuple(sorted(kw.items()))
    if key not in _NC_CACHE:
        _NC_CACHE[key] = build(**kw)
    return _NC_CACHE[key]


def kernel(a, beta):
    """Full-input entry point: a [16,128,256,256] f32, beta [1] f32."""
    a = np.ascontiguousarray(np.asarray(a, dtype=np.float32))
    beta = np.asarray(beta, dtype=np.float32)
    nb, ch, h, w = a.shape
    n = h * w
    s = nb // N_CORES
    a3 = a.reshape(nb, ch, n)
    beta_b = np.broadcast_to(beta.reshape(1, 1), (ch, 1)).copy()

    nc = _get_nc(S=s, C=ch, N=n)
    in_maps = [
        {"a": a3[i * s:(i + 1) * s], "beta": beta_b} for i in range(N_CORES)
    ]
    res = run_bass_kernel_spmd(nc, in_maps, list(range(N_CORES)))
    out = np.concatenate([res.results[i]["out"] for i in range(N_CORES)], axis=0)
    return out.reshape(nb, ch, h, w).astype(np.float32, copy=False)


# revision 38
# speedup vs baseline: 1.1718x; 1.0732x over previous
"""Trainium2 Bass kernel for a channel-attention block.

Reference math (per batch sample, a: [C, N] with C=128 channels,
N = H*W spatial):
    b   = a @ a.T                  # [C, C] channel affinity (Gram)
    x   = softmax(b, axis=-1)
    c   = x @ a                    # [C, N]
    out = beta * c + a

Sharding: data-parallel over the batch dim — 16 samples / 8 cores =
2 samples per NeuronCore, no cross-core communication.

Per-core pipeline (per sample):
  stage A: stream `a` in [128, 2048] f32 tiles; PE-transpose each
           128x128 block (n on partitions), ACT copy-casts PSUM->SBUF
           bf16, then bf16 Gram matmuls accumulate b in one PSUM bank.
  stage B: row softmax on b (DVE max, ACT exp(+bias) with fused row
           sum, DVE reciprocal, DVE scale), PE-transpose X -> XT.
  stage C: stream `a` again; c = XT.T @ a via fp32 matmuls (no cast
           pass needed; PE has headroom under the DMA roofline), fused
           DVE epilogue out = (c * beta) + a, store.

The kernel is DMA-bound: 3 passes x 32 MB per sample = 192 MB per
core at ~360 GB/s.
"""

import numpy as np

import concourse.bass as bass
import concourse.mybir as mybir
import concourse.tile as tile
from concourse import bacc
from concourse.bass_utils import run_bass_kernel_spmd
from concourse.masks import make_identity

F32 = mybir.dt.float32
F32R = mybir.dt.float32r
BF16 = mybir.dt.bfloat16

N_CORES = 8
B, C, H, W = 16, 128, 256, 256
N_FULL = H * W
S = B // N_CORES  # samples per core


def build(S=S, C=C, N=N_FULL, load=2048, stage_c="bf16", bufs=3, cache_k=14,
          precast=True, eng_precast="dve", eng_atcopy="alt", eng_ccast="dve",
          split_load_rings=False, alt2rings=False):
    """Build + compile the per-core Bass program.

    Emission interleaves sample s's stage C with sample s+1's stage A so
    the (DMA-bound) attend phase overlaps the (PE-heavier) Gram phase;
    stage A is emitted at 2x rate so its Gram finishes (and softmax runs)
    while C(s-1) still has work, hiding the softmax latency.
    The last `cache_k/2` stage-A input tiles of each sample stay resident
    in SBUF (two pools, alternating by sample parity, so slot reuse never
    stalls the next sample's fills) and stage C skips re-loading them.
    """
    assert C == 128 and N % load == 0 and load % 512 == 0
    cache_k = min(cache_k, N // load)
    nc = bacc.Bacc("TRN2", target_bir_lowering=False, debug=False)

    a_d = nc.dram_tensor("a", [S, C, N], F32, kind="ExternalInput").ap()
    beta_d = nc.dram_tensor("beta", [C, 1], F32, kind="ExternalInput").ap()
    out_d = nc.dram_tensor("out", [S, C, N], F32, kind="ExternalOutput").ap()

    n_loads = N // load
    TW = 512             # transpose-group width: 4 transposes per ACT copy
    MM_N = 512           # stage-C matmul moving width (1 PSUM bank f32)
    n_gram_mm = N // 128

    with tile.TileContext(nc) as tc:
        with (
            tc.tile_pool(name="const", bufs=1) as const_pool,
            tc.tile_pool(name="a_in", bufs=bufs) as a_pool,
            tc.tile_pool(name="at", bufs=4) as at_pool,
            tc.tile_pool(name="sm", bufs=2) as sm_pool,
            tc.tile_pool(name="cin", bufs=bufs) as cin_pool,
            tc.tile_pool(name="cout", bufs=bufs) as cout_pool,
            tc.tile_pool(name="acache", bufs=max(cache_k, 1)) as cache_pool,
            tc.tile_pool(name="tp_ps", bufs=3, space="PSUM") as tp_psum,
            tc.tile_pool(name="gram_ps", bufs=2, space="PSUM") as gram_psum,
            tc.tile_pool(name="c_ps", bufs=3, space="PSUM") as c_psum,
        ):
            ident = const_pool.tile([128, 128], F32, tag="ident")
            make_identity(nc, ident)
            ident_bf = const_pool.tile([128, 128], BF16, tag="identbf")
            make_identity(nc, ident_bf)
            beta_sb = const_pool.tile([C, 1], F32, tag="beta")
            nc.sync.dma_start(beta_sb, beta_d)

            def copy_op(engine_sel, idx, out, in_):
                """Route a copy/cast to ACT or DVE per engine_sel."""
                if engine_sel == "act" or (engine_sel == "alt" and idx % 2 == 0):
                    nc.scalar.copy(out, in_)
                else:
                    nc.vector.tensor_copy(out, in_)

            gram_state = {}   # s -> (b_ps, mm_count)
            xt_w = {}         # s -> lhsT weights for stage C
            beta_row = {}     # s -> beta/rowsum [C,1]
            cached = {}       # (s, j) -> SBUF-resident a tile

            def stage_a_chunk(s, j):
                """Load tile j of sample s, transpose, Gram-accumulate."""
                if s not in gram_state:
                    b_ps = gram_psum.tile([C, C], F32, tag="gram",
                                          name=f"gram_{s}")
                    gram_state[s] = [b_ps, 0]
                st = gram_state[s]
                b_ps = st[0]
                if j >= n_loads - cache_k:
                    a_t = cache_pool.tile([C, load], F32, tag="acache",
                                          name=f"ac_{s}_{j}")
                    cached[(s, j)] = a_t
                else:
                    a_t = a_pool.tile([C, load], F32, tag="a_in",
                                      name=f"a_{s}_{j}")
                if alt2rings:
                    ld = nc.scalar if j % 2 else nc.sync
                else:
                    ld = nc.gpsimd if (split_load_rings and j % 2) else nc.sync
                ld.dma_start(a_t, a_d[s, :, j * load:(j + 1) * load])
                for g in range(load // TW):
                    src = a_t[:, g * TW:(g + 1) * TW]
                    if precast:
                        abf = at_pool.tile([128, TW], BF16, tag="abf",
                                           name=f"abf_{s}_{j}_{g}")
                        copy_op(eng_precast, g, abf, src)
                        src = abf
                    t_dt = BF16 if precast else F32
                    t_id = ident_bf if precast else ident
                    tp = tp_psum.tile([128, TW], t_dt, tag="tp",
                                      name=f"tp_{s}_{j}_{g}")
                    for q in range(TW // 128):
                        nc.tensor.transpose(
                            tp[:, q * 128:(q + 1) * 128],
                            src[:, q * 128:(q + 1) * 128],
                            t_id,
                        )
                    at_t = at_pool.tile([128, TW], BF16, tag="at",
                                        name=f"at_{s}_{j}_{g}")
                    copy_op(eng_atcopy, g, at_t, tp)
                    for q in range(TW // 128):
                        st[1] += 1
                        nc.tensor.matmul(
                            b_ps,
                            lhsT=at_t[:, q * 128:(q + 1) * 128],
                            rhs=at_t[:, q * 128:(q + 1) * 128],
                            start=(st[1] == 1),
                            stop=(st[1] == n_gram_mm),
                        )

            def softmax(s):
                """Unnormalized softmax: E = exp(b - rowmax), transposed for
                stage C. The 1/rowsum normalization folds into the epilogue
                scalar bs = beta/rowsum, keeping the first stage-C matmul off
                the DVE reciprocal's critical path."""
                b_ps = gram_state[s][0]
                negm = sm_pool.tile([C, 1], F32, tag="negm", name=f"negm_{s}")
                nc.vector.tensor_reduce(
                    negm, b_ps, axis=mybir.AxisListType.X,
                    op=mybir.AluOpType.max, negate=True,
                )
                e_dt = F32 if stage_c == "f32" else BF16
                e_t = sm_pool.tile([C, C], e_dt, tag="e", name=f"e_{s}")
                ssum = sm_pool.tile([C, 1], F32, tag="ssum", name=f"ssum_{s}")
                nc.scalar.activation(
                    e_t, b_ps, mybir.ActivationFunctionType.Exp,
                    bias=negm, accum_out=ssum,
                )
                xt_ps = tp_psum.tile([128, TW], e_dt, tag="tp", name=f"xtp_{s}")
                nc.tensor.transpose(xt_ps[:, :128], e_t,
                                    ident if e_dt == F32 else ident_bf)
                xt_sb = sm_pool.tile([C, C], e_dt, tag="xt", name=f"xt_{s}")
                nc.scalar.copy(xt_sb, xt_ps[:, :128])
                xt_w[s] = xt_sb
                rec = sm_pool.tile([C, 1], F32, tag="rec", name=f"rec_{s}")
                nc.vector.reciprocal(rec, ssum)
                bs = sm_pool.tile([C, 1], F32, tag="bs", name=f"bs_{s}")
                nc.vector.tensor_scalar_mul(bs, rec, beta_sb)
                beta_row[s] = bs

            def stage_c_chunk(s, j):
                """c = XT.T @ a on tile j; out = beta*c + a; store."""
                lhs_w = xt_w[s]
                c_in = cached.pop((s, j), None)
                if c_in is None:
                    c_in = cin_pool.tile([C, load], F32, tag="cin",
                                         name=f"cin_{s}_{j}")
                    if alt2rings:
                        ld = nc.scalar if j % 2 else nc.sync
                    else:
                        ld = nc.gpsimd if (split_load_rings and j % 2) else nc.sync
                    ld.dma_start(c_in, a_d[s, :, j * load:(j + 1) * load])
                if stage_c == "bf16":
                    rhs_b = cin_pool.tile([C, load], BF16, tag="cinb",
                                          name=f"cinb_{s}_{j}")
                    copy_op(eng_ccast, j, rhs_b, c_in)
                c_out = cout_pool.tile([C, load], F32, tag="cout",
                                       name=f"cout_{s}_{j}")
                for q in range(load // MM_N):
                    sl = slice(q * MM_N, (q + 1) * MM_N)
                    rhs_mm = rhs_b[:, sl] if stage_c == "bf16" else c_in[:, sl]
                    c_ps = c_psum.tile([128, MM_N], F32, tag="cps",
                                       name=f"cps_{s}_{j}_{q}")
                    nc.tensor.matmul(
                        c_ps, lhsT=lhs_w, rhs=rhs_mm, start=True, stop=True,
                    )
                    nc.vector.scalar_tensor_tensor(
                        out=c_out[:, sl],
                        in0=c_ps,
                        scalar=beta_row[s],
                        in1=c_in[:, sl],
                        op0=mybir.AluOpType.mult,
                        op1=mybir.AluOpType.add,
                    )
                st = (nc.sync if j % 2 else nc.scalar) if alt2rings else nc.scalar
                st.dma_start(out_d[s, :, j * load:(j + 1) * load], c_out)

            # Stage C consumes its SBUF-cached tiles FIRST so the shared
            # cache slots free before the next sample's stage-A tail needs
            # to refill them.
            def c_order():
                return (list(range(n_loads - cache_k, n_loads))
                        + list(range(0, n_loads - cache_k)))

            # Software-pipelined emission across samples:
            #   A(0); sm(0); [A(1) 2x-rate x C(0)]; sm(1); C(0) tail; ...
            # Stage A runs at 2x emission rate so its Gram (and softmax)
            # complete while C(s-1) still has work queued.
            for j in range(n_loads):
                stage_a_chunk(0, j)
            softmax(0)
            for s in range(1, S):
                order = c_order()
                ci = 0
                for j in range(n_loads):
                    stage_a_chunk(s, j)
                    if j % 2 == 1:
                        stage_c_chunk(s - 1, order[ci])
                        ci += 1
                softmax(s)
                while ci < n_loads:
                    stage_c_chunk(s - 1, order[ci])
                    ci += 1
            for j in c_order():
                stage_c_chunk(S - 1, j)

    nc.compile()
    return nc


_NC_CACHE: dict = {}


def _get_nc(**kw):
    key = tuple(sorted(kw.items()))
    if key not in _NC_CACHE:
        _NC_CACHE[key] = build(**kw)
    return _NC_CACHE[key]


def kernel(a, beta):
    """Full-input entry point: a [16,128,256,256] f32, beta [1] f32."""
    a = np.ascontiguousarray(np.asarray(a, dtype=np.float32))
    beta = np.asarray(beta, dtype=np.float32)
    nb, ch, h, w = a.shape
    n = h * w
    s = nb // N_CORES
    a3 = a.reshape(nb, ch, n)
    beta_b = np.broadcast_to(beta.reshape(1, 1), (ch, 1)).copy()

    nc = _get_nc(S=s, C=ch, N=n)
    in_maps = [
        {"a": a3[i * s:(i + 1) * s], "beta": beta_b} for i in range(N_CORES)
    ]
    res = run_bass_kernel_spmd(nc, in_maps, list(range(N_CORES)))
    out = np.concatenate([res.results[i]["out"] for i in range(N_CORES)], axis=0)
    return out.reshape(nb, ch, h, w).astype(np.float32, copy=False)
